# revision 9
# baseline (speedup 1.0000x reference)
"""Trainium2 Bass kernel for nn_MultiHeadedAttention_257698038597.

Multi-headed attention with channels: query/key/value [B=2,T=512,C=8,D=512],
mask [B,T,T,1]; four Linear(512,512) layers. Sharding: data-parallel over the
16 (b,c) pairs -> 2 units per core across 8 cores.

v2 design (per-core SPMD; bf16 operand path, rel err ~8e-3):
  - host pre-transposes activations to x^T [D,T] bf16, mask to 0/1 bf16 in
    [s,t] orientation, folds the v-bias into the output bias (softmax rows
    sum to 1): bo' = bv @ Wo + bo. All matmul operands bf16 (PSUM f32).
  - projections: qT/kT = Wq^T x + bq with the bias applied on the ACT engine
    (per-partition bias) during PSUM eviction; v natural layout with a ones
    column per head (66-stride blocks), evicted on DVE.
  - scores^T[s,t] per head, K=64 head pairs packed on partition halves. NO
    mask preload on PE: exp(scores/8) on ACT -> bf16, then a single DVE
    tensor_tensor multiply with the 0/1 mask (stride-0 broadcast across the
    j dim, 2x DVE mode) zeroes masked entries exactly.
  - att@v with the ones column producing the (masked) softmax normalizer as
    psum row 64; per head pair: 2 DVE copies of the sums rows into a [8,T]
    tile, one batched reciprocal, 2 gpsimd partition-broadcasts, 2 DVE
    multiplies into attT (bf16).
  - y = attT^T Wo + bo' (DVE tensor_tensor on eviction), one merged store
    DMA per unit.
  - the two units' attention g-loops are interleaved so PE score/av matmuls
    of one unit hide the ACT exp latency of the other.
  - DMAs: merged whole-tensor transfers; weights on the gpsimd (SWDGE) queue,
    activations + stores on sync (HWDGE), keeping the ACT queue DMA-free.
"""
import numpy as np

import concourse.bass as bass
import concourse.mybir as mybir
import concourse.tile as tile
from concourse import bacc
from concourse.bass import ts

P = 128
B, T, C, D = 2, 512, 8, 512
H, DK = 8, 64
KO = D // P             # 4 contraction chunks
U = 2                   # units (b,c pairs) per core
VS = 66                 # v_sb per-head stride: 64 v cols + 1 ones + 1 pad
NCORES = 8

F32 = mybir.dt.float32
BF16 = mybir.dt.bfloat16

EXP = mybir.ActivationFunctionType.Exp
IDENT = mybir.ActivationFunctionType.Identity
MUL = mybir.AluOpType.mult
ADD = mybir.AluOpType.add


def build_nc(repeat=1):
    nc = bacc.Bacc("TRN2", target_bir_lowering=False, debug=False)

    xqt = nc.dram_tensor("xqt", [U, P, KO, T], BF16, kind="ExternalInput")
    xkt = nc.dram_tensor("xkt", [U, P, KO, T], BF16, kind="ExternalInput")
    xvt = nc.dram_tensor("xvt", [U, P, KO, T], BF16, kind="ExternalInput")
    # 0/1 mask, transposed: m01[p, so, t] = mask[b, t, so*128+p]
    m01 = nc.dram_tensor("m01", [P, KO, T], BF16, kind="ExternalInput")
    wq = nc.dram_tensor("wq", [P, KO, D], BF16, kind="ExternalInput")
    wk = nc.dram_tensor("wk", [P, KO, D], BF16, kind="ExternalInput")
    wv = nc.dram_tensor("wv", [P, KO, D], BF16, kind="ExternalInput")
    wo = nc.dram_tensor("wo", [P, KO, D], BF16, kind="ExternalInput")
    bqd = nc.dram_tensor("bqd", [P, KO], F32, kind="ExternalInput")
    bkd = nc.dram_tensor("bkd", [P, KO], F32, kind="ExternalInput")
    bo2d = nc.dram_tensor("bo2d", [1, D], F32, kind="ExternalInput")
    y = nc.dram_tensor("y", [U, P, KO, D], F32, kind="ExternalOutput")

    with tile.TileContext(nc) as tc:
        import contextlib
        with contextlib.ExitStack() as ctx:
            const = ctx.enter_context(tc.tile_pool(name="const", bufs=1))
            xt_pool = ctx.enter_context(tc.tile_pool(name="xt", bufs=2))
            qk_pool = ctx.enter_context(tc.tile_pool(name="qk", bufs=2))
            p_pool = ctx.enter_context(tc.tile_pool(name="pp", bufs=8))
            att_pool = ctx.enter_context(tc.tile_pool(name="att", bufs=2))
            nrm_pool = ctx.enter_context(tc.tile_pool(name="nrm", bufs=2))
            y_pool = ctx.enter_context(tc.tile_pool(name="y", bufs=2))
            # [128,512] f32 ring shared by proj/av/outproj psums (1 bank each)
            ps_a = ctx.enter_context(tc.tile_pool(name="psa", bufs=2, space="PSUM"))
            # [128,2,512] f32 scores psums (2 banks each)
            ps_sc = ctx.enter_context(tc.tile_pool(name="pssc", bufs=3, space="PSUM"))

            # ---- constants (gpsimd/SWDGE queue: off the HWDGE ring)
            wq_sb = const.tile([P, KO, D], BF16, tag="wq")
            wk_sb = const.tile([P, KO, D], BF16, tag="wk")
            wv_sb = const.tile([P, KO, D], BF16, tag="wv")
            wo_sb = const.tile([P, KO, D], BF16, tag="wo")
            m01_sb = const.tile([P, KO, T], BF16, tag="m01")
            bq_sb = const.tile([P, KO], F32, tag="bq")
            bk_sb = const.tile([P, KO], F32, tag="bk")
            bo2_sb = const.tile([1, D], F32, tag="bo2")
            bo2_bc = const.tile([P, D], F32, tag="bo2bc")

            # wq split in two so the first projections start sooner
            nc.gpsimd.dma_start(out=wq_sb[:, 0:2, :], in_=wq[:, 0:2, :])
            nc.gpsimd.dma_start(out=wq_sb[:, 2:4, :], in_=wq[:, 2:4, :])
            nc.gpsimd.dma_start(out=bq_sb, in_=bqd[:, :])
            nc.gpsimd.dma_start(out=bk_sb, in_=bkd[:, :])
            nc.gpsimd.dma_start(out=wk_sb, in_=wk[:, :, :])
            nc.gpsimd.dma_start(out=wv_sb, in_=wv[:, :, :])
            nc.gpsimd.dma_start(out=m01_sb, in_=m01[:, :, :])
            nc.gpsimd.dma_start(out=wo_sb, in_=wo[:, :, :])
            nc.gpsimd.dma_start(out=bo2_sb, in_=bo2d[:, :])
            nc.gpsimd.partition_broadcast(bo2_bc[:], bo2_sb[0:1, :])
            # preload the ACT exp table while DMAs are in flight
            warm = const.tile([1, 8], F32, tag="warm")
            nc.vector.memset(warm[:], 1.0)
            nc.scalar.activation(warm[:], warm[:], EXP, scale=0.125)

            def m01_bcast_view(so):
                # [128, (0-stride j=2), 512] broadcast view of m01_sb[:, so, :]
                return bass.AP(
                    tensor=m01_sb.tensor, offset=m01_sb[:, so, 0].offset,
                    ap=[list(m01_sb[:].ap[0]), [0, 2], [1, T]])

            for _rep in range(repeat):
                # ---- activation loads (sync/HWDGE queue)
                x_sbs = []
                for u in range(U):
                    xq_sb = xt_pool.tile([P, KO, T], BF16, tag="xq")
                    xk_sb = xt_pool.tile([P, KO, T], BF16, tag="xk")
                    xv_sb = xt_pool.tile([P, KO, T], BF16, tag="xv")
                    if u == 0:
                        # split the first xq so matmuls start sooner
                        nc.sync.dma_start(out=xq_sb[:, 0:2, :], in_=xqt[u, :, 0:2, :])
                        nc.sync.dma_start(out=xq_sb[:, 2:4, :], in_=xqt[u, :, 2:4, :])
                    else:
                        nc.sync.dma_start(out=xq_sb[:, :, :], in_=xqt[u, :, :, :])
                    nc.sync.dma_start(out=xk_sb[:, :, :], in_=xkt[u, :, :, :])
                    nc.sync.dma_start(out=xv_sb[:, :, :], in_=xvt[u, :, :, :])
                    x_sbs.append((xq_sb, xk_sb, xv_sb))

                # ---- projections
                qkv = []
                for u in range(U):
                    xq_sb, xk_sb, xv_sb = x_sbs[u]
                    qT_sb = qk_pool.tile([P, KO, T], BF16, tag="qT")
                    kT_sb = qk_pool.tile([P, KO, T], BF16, tag="kT")
                    v_sb = qk_pool.tile([P, KO, H * VS], BF16, tag="v")
                    # ones columns (position 64 of each 66-wide head block)
                    ones_view = bass.AP(
                        tensor=v_sb.tensor, offset=v_sb[:, 0, 64].offset,
                        ap=[list(v_sb[:].ap[0]), [H * VS, KO], [VS, H], [1, 1]])
                    nc.vector.memset(ones_view, 1.0)

                    for mo in range(KO):
                        psq = ps_a.tile([P, T], F32, tag="ps")
                        for ko in range(KO):
                            nc.tensor.matmul(psq[:], wq_sb[:, ko, ts(mo, P)],
                                             xq_sb[:, ko, :],
                                             start=(ko == 0), stop=(ko == KO - 1))
                        nc.scalar.activation(qT_sb[:, mo, :], psq[:], IDENT,
                                             bias=bq_sb[:, mo, None])
                    for mo in range(KO):
                        psk = ps_a.tile([P, T], F32, tag="ps")
                        for ko in range(KO):
                            nc.tensor.matmul(psk[:], wk_sb[:, ko, ts(mo, P)],
                                             xk_sb[:, ko, :],
                                             start=(ko == 0), stop=(ko == KO - 1))
                        nc.scalar.activation(kT_sb[:, mo, :], psk[:], IDENT,
                                             bias=bk_sb[:, mo, None])
                    for mo in range(KO):
                        psv = ps_a.tile([P, T], F32, tag="ps")
                        for ko in range(KO):
                            nc.tensor.matmul(psv[:], xv_sb[:, ko, ts(mo, P)],
                                             wv_sb[:, ko, :],
                                             start=(ko == 0), stop=(ko == KO - 1))
                        # scatter into per-head 66-strided blocks: [P, H, DK]
                        v_dst = bass.AP(
                            tensor=v_sb.tensor, offset=v_sb[:, mo, 0].offset,
                            ap=[list(v_sb[:].ap[0]), [VS, H], [1, DK]])
                        nc.vector.tensor_copy(out=v_dst,
                                              in_=psv[:].rearrange("p (h d) -> p h d", h=H))
                    qkv.append((qT_sb, kT_sb, v_sb))

                # ---- attention, units interleaved: scores/exp/mask S(u,g),
                # av+normalize A(u,g); PE work of one unit hides ACT exp of
                # the other.
                p_tiles = {}
                att_units = []
                for u in range(U):
                    att_units.append(
                        [att_pool.tile([P, T], BF16, tag=f"attT{g}",
                                       name=f"attT{g}_u{u}")
                         for g in range(4)])

                def S(u, g):
                    qT_sb, kT_sb, v_sb = qkv[u]
                    for so in range(KO):
                        sc = ps_sc.tile([P, 2, T], F32, tag="sc")
                        for j in range(2):
                            lo = 64 * j
                            nc.tensor.matmul(
                                sc[:, j, :],
                                kT_sb[lo:lo + 64, g, ts(so, P)],
                                qT_sb[lo:lo + 64, g, :],
                                start=True, stop=True)
                        pt = p_pool.tile([P, 2, T], BF16, tag="p")
                        nc.scalar.activation(pt[:], sc[:], EXP, scale=0.125)
                        # zero masked entries exactly (0/1 mask, both heads)
                        nc.vector.tensor_tensor(out=pt[:], in0=pt[:],
                                                in1=m01_bcast_view(so), op=MUL)
                        p_tiles[(u, g, so)] = pt

                def A(u, g):
                    qT_sb, kT_sb, v_sb = qkv[u]
                    attT_g = att_units[u][g]
                    # pair sums at partition 0 (partition_broadcast requires
                    # partition-0 source AND destination on hardware)
                    s2 = nrm_pool.tile([1, 2, T], F32, tag="s2")
                    avs = []
                    for j, h in enumerate((2 * g, 2 * g + 1)):
                        av = ps_a.tile([P, T], F32, tag="ps", name=f"av{u}{g}{j}")
                        for so in range(KO):
                            nc.tensor.matmul(
                                av[0:65, :],
                                v_sb[:, so, VS * h:VS * h + 65],
                                p_tiles[(u, g, so)][:, j, :],
                                start=(so == 0), stop=(so == KO - 1))
                        nc.scalar.copy(out=s2[0:1, j, :], in_=av[64:65, :])
                        avs.append(av)
                    # one batched in-place reciprocal for the head pair
                    nc.vector.reciprocal_approx_fast(out=s2[:], in_=s2[:])
                    for j in range(2):
                        lo = 64 * j
                        bc = nrm_pool.tile([64, T], F32, tag=f"bc{j}")
                        nc.gpsimd.partition_broadcast(bc[:], s2[0:1, j, :])
                        nc.vector.tensor_tensor(
                            out=attT_g[lo:lo + 64, :],
                            in0=avs[j][0:64, :], in1=bc[:], op=MUL)

                # A(u,g) is emitted right after S(other,g) so av matmuls are
                # never queued behind a scores block stalled on the sc ring.
                sched = [(0, 0, 'S'), (1, 0, 'S'), (0, 0, 'A'), (0, 1, 'S'),
                         (1, 0, 'A'), (1, 1, 'S'), (0, 1, 'A'), (0, 2, 'S'),
                         (1, 1, 'A'), (1, 2, 'S'), (0, 2, 'A'), (0, 3, 'S'),
                         (1, 2, 'A'), (1, 3, 'S'), (0, 3, 'A'), (1, 3, 'A')]
                for u, g, kind in sched:
                    (S if kind == 'S' else A)(u, g)

                # ---- output projections + merged store per unit
                for u in range(U):
                    attT_k = att_units[u]
                    y_sb = y_pool.tile([P, KO, D], F32, tag="y")
                    for tc_i in range(KO):
                        psy = ps_a.tile([P, T], F32, tag="ps")
                        for ko in range(KO):
                            nc.tensor.matmul(psy[:], attT_k[ko][:, ts(tc_i, P)],
                                             wo_sb[:, ko, :],
                                             start=(ko == 0), stop=(ko == KO - 1))
                        nc.vector.tensor_tensor(out=y_sb[:, tc_i, :], in0=psy[:],
                                                in1=bo2_bc[:], op=ADD)
                    # store on the gpsimd/SWDGE queue so next-iteration x
                    # loads on sync are not blocked behind the store's wait
                    nc.gpsimd.dma_start(out=y[u, :, :, :], in_=y_sb[:])

    nc.compile()
    return nc


_NC_CACHE = {}


def _get_nc(repeat=1):
    if repeat not in _NC_CACHE:
        _NC_CACHE[repeat] = build_nc(repeat)
    return _NC_CACHE[repeat]


def _bf16():
    import ml_dtypes
    return ml_dtypes.bfloat16


def _chunkT(x):
    """[T,D] -> x^T chunked [P, KO, T] (din = ko*128 + p)."""
    return np.ascontiguousarray(x.T.reshape(KO, P, T).transpose(1, 0, 2))


def _chunkW(w):
    """[D,D] (in,out) -> [P, KO, D]."""
    return np.ascontiguousarray(w.reshape(KO, P, D).transpose(1, 0, 2))


def make_in_maps(query, key, value, mask, Wq, bq, Wk, bk, Wv, bv, Wo, bo):
    bf16 = _bf16()
    query = np.asarray(query, np.float32)
    key = np.asarray(key, np.float32)
    value = np.asarray(value, np.float32)
    mask = np.asarray(mask)
    Wq, Wk, Wv, Wo = (np.asarray(w, np.float32) for w in (Wq, Wk, Wv, Wo))
    bq, bk, bv, bo = (np.asarray(b, np.float32) for b in (bq, bk, bv, bo))

    bo2 = (bv @ Wo + bo).astype(np.float32).reshape(1, D)
    wq_h = _chunkW(Wq).astype(bf16)
    wk_h = _chunkW(Wk).astype(bf16)
    wv_h = _chunkW(Wv).astype(bf16)
    wo_h = _chunkW(Wo).astype(bf16)
    bq_h = np.ascontiguousarray(bq.reshape(KO, P).T)
    bk_h = np.ascontiguousarray(bk.reshape(KO, P).T)

    in_maps = []
    for core in range(NCORES):
        b = core // 4
        cs = [2 * (core % 4), 2 * (core % 4) + 1]
        xq = np.stack([_chunkT(query[b, :, c, :]) for c in cs]).astype(bf16)
        xk = np.stack([_chunkT(key[b, :, c, :]) for c in cs]).astype(bf16)
        xv = np.stack([_chunkT(value[b, :, c, :]) for c in cs]).astype(bf16)
        # 0/1 mask, transposed to [s-part, so, t]
        mb = _chunkT(mask[b, :, :, 0].astype(np.float32)).astype(bf16)
        mb = np.ascontiguousarray(mb)
        in_maps.append({
            "xqt": xq, "xkt": xk, "xvt": xv, "m01": mb,
            "wq": wq_h, "wk": wk_h, "wv": wv_h, "wo": wo_h,
            "bqd": bq_h, "bkd": bk_h, "bo2d": bo2,
        })
    return in_maps


def assemble(results):
    out = np.empty((B, T, C, D), np.float32)
    for core, res in enumerate(results):
        b = core // 4
        cs = [2 * (core % 4), 2 * (core % 4) + 1]
        yv = res["y"]  # [U, P, KO, D]; t = ko*128 + p
        for u, c in enumerate(cs):
            out[b, :, c, :] = yv[u].transpose(1, 0, 2).reshape(T, D)
    return out


def kernel(**inputs):
    from concourse.bass_utils import run_bass_kernel_spmd
    nc = _get_nc()
    in_maps = make_in_maps(**inputs)
    res = run_bass_kernel_spmd(nc, in_maps, core_ids=list(range(NCORES)))
    return assemble(res.results)


if __name__ == "__main__":
    # single-core sim check of core 0 (units b=0, c=0,1)
    import jax
    jax.config.update("jax_platforms", "cpu")
    import sys
    sys.path.insert(0, "/root/problem")
    import reference

    inp = {k: np.asarray(v) for k, v in reference.setup_inputs().items()}
    ref = np.asarray(reference.reference(**inp))

    from concourse.bass_interp import CoreSim
    nc = _get_nc()
    in_maps = make_in_maps(**inp)
    core = 0
    sim = CoreSim(nc)
    sim.assign_tensors(in_maps[core])
    sim.simulate()
    yv = sim.tensor("y")
    b = core // 4
    cs = [2 * (core % 4), 2 * (core % 4) + 1]
    for u, c in enumerate(cs):
        got = yv[u].transpose(1, 0, 2).reshape(T, D)
        want = ref[b, :, c, :]
        err = np.abs(got - want)
        print(f"core0 unit{u} (b={b},c={c}): absmax={err.max():.3e} "
              f"rel={err.max()/np.abs(want).max():.3e}")


# revision 12
# speedup vs baseline: 3.5495x; 3.5495x over previous
"""Trainium2 Bass kernel for nn_MultiHeadedAttention_257698038597.

Multi-headed attention with channels: query/key/value [B=2,T=512,C=8,D=512],
mask [B,T,T,1]; four Linear(512,512) layers. Sharding: data-parallel over the
16 (b,c) pairs -> 2 units per core across 8 cores.

v2 design (per-core SPMD; bf16 operand path, rel err ~8e-3):
  - host pre-transposes activations to x^T [D,T] bf16, mask to 0/1 bf16 in
    [s,t] orientation, folds the v-bias into the output bias (softmax rows
    sum to 1): bo' = bv @ Wo + bo. All matmul operands bf16 (PSUM f32).
  - projections: qT/kT = Wq^T x + bq with the bias applied on the ACT engine
    (per-partition bias) during PSUM eviction; v natural layout with a ones
    column per head (66-stride blocks), evicted on DVE.
  - scores^T[s,t] per head, K=64 head pairs packed on partition halves. NO
    mask preload on PE: exp(scores/8) on ACT -> bf16, then a single DVE
    tensor_tensor multiply with the 0/1 mask (stride-0 broadcast across the
    j dim, 2x DVE mode) zeroes masked entries exactly.
  - att@v with the ones column producing the (masked) softmax normalizer as
    psum row 64; per head pair: 2 DVE copies of the sums rows into a [8,T]
    tile, one batched reciprocal, 2 gpsimd partition-broadcasts, 2 DVE
    multiplies into attT (bf16).
  - y = attT^T Wo + bo' (DVE tensor_tensor on eviction), one merged store
    DMA per unit.
  - the two units' attention g-loops are interleaved so PE score/av matmuls
    of one unit hide the ACT exp latency of the other.
  - DMAs: merged whole-tensor transfers; weights on the gpsimd (SWDGE) queue,
    activations + stores on sync (HWDGE), keeping the ACT queue DMA-free.
"""
import numpy as np

import concourse.bass as bass
import concourse.mybir as mybir
import concourse.tile as tile
from concourse import bacc
from concourse.bass import ts

P = 128
B, T, C, D = 2, 512, 8, 512
H, DK = 8, 64
KO = D // P             # 4 contraction chunks
U = 2                   # units (b,c pairs) per core
VS = 66                 # v_sb per-head stride: 64 v cols + 1 ones + 1 pad
NCORES = 8

F32 = mybir.dt.float32
BF16 = mybir.dt.bfloat16

EXP = mybir.ActivationFunctionType.Exp
IDENT = mybir.ActivationFunctionType.Identity
MUL = mybir.AluOpType.mult
ADD = mybir.AluOpType.add


def build_nc(repeat=1):
    nc = bacc.Bacc("TRN2", target_bir_lowering=False, debug=False)

    xqt = nc.dram_tensor("xqt", [U, P, KO, T], BF16, kind="ExternalInput")
    xkt = nc.dram_tensor("xkt", [U, P, KO, T], BF16, kind="ExternalInput")
    xvt = nc.dram_tensor("xvt", [U, P, KO, T], BF16, kind="ExternalInput")
    # 0/1 mask, transposed: m01[p, so, t] = mask[b, t, so*128+p]
    m01 = nc.dram_tensor("m01", [P, KO, T], BF16, kind="ExternalInput")
    wq = nc.dram_tensor("wq", [P, KO, D], BF16, kind="ExternalInput")
    wk = nc.dram_tensor("wk", [P, KO, D], BF16, kind="ExternalInput")
    wv = nc.dram_tensor("wv", [P, KO, D], BF16, kind="ExternalInput")
    wo = nc.dram_tensor("wo", [P, KO, D], BF16, kind="ExternalInput")
    bqd = nc.dram_tensor("bqd", [P, KO], F32, kind="ExternalInput")
    bkd = nc.dram_tensor("bkd", [P, KO], F32, kind="ExternalInput")
    bo2d = nc.dram_tensor("bo2d", [1, D], F32, kind="ExternalInput")
    y = nc.dram_tensor("y", [U, P, KO, D], F32, kind="ExternalOutput")

    with tile.TileContext(nc) as tc:
        import contextlib
        with contextlib.ExitStack() as ctx:
            const = ctx.enter_context(tc.tile_pool(name="const", bufs=1))
            xt_pool = ctx.enter_context(tc.tile_pool(name="xt", bufs=2))
            qk_pool = ctx.enter_context(tc.tile_pool(name="qk", bufs=2))
            p_pool = ctx.enter_context(tc.tile_pool(name="pp", bufs=8))
            att_pool = ctx.enter_context(tc.tile_pool(name="att", bufs=2))
            nrm_pool = ctx.enter_context(tc.tile_pool(name="nrm", bufs=2))
            y_pool = ctx.enter_context(tc.tile_pool(name="y", bufs=2))
            # [128,512] f32 ring shared by proj/av/outproj psums (1 bank each);
            # bufs=4 so the av->normalizer tail never throttles new av groups
            ps_a = ctx.enter_context(tc.tile_pool(name="psa", bufs=4, space="PSUM"))
            # [128,2,512] f32 scores psums (2 banks each)
            ps_sc = ctx.enter_context(tc.tile_pool(name="pssc", bufs=2, space="PSUM"))

            # ---- constants (gpsimd/SWDGE queue: off the HWDGE ring)
            wq_sb = const.tile([P, KO, D], BF16, tag="wq")
            wk_sb = const.tile([P, KO, D], BF16, tag="wk")
            wv_sb = const.tile([P, KO, D], BF16, tag="wv")
            wo_sb = const.tile([P, KO, D], BF16, tag="wo")
            m01_sb = const.tile([P, KO, T], BF16, tag="m01")
            bq_sb = const.tile([P, KO], F32, tag="bq")
            bk_sb = const.tile([P, KO], F32, tag="bk")
            bo2_sb = const.tile([1, D], F32, tag="bo2")
            bo2_bc = const.tile([P, D], F32, tag="bo2bc")

            # wq split in two so the first projections start sooner
            nc.gpsimd.dma_start(out=wq_sb[:, 0:2, :], in_=wq[:, 0:2, :])
            nc.gpsimd.dma_start(out=wq_sb[:, 2:4, :], in_=wq[:, 2:4, :])
            nc.gpsimd.dma_start(out=bq_sb, in_=bqd[:, :])
            nc.gpsimd.dma_start(out=bk_sb, in_=bkd[:, :])
            nc.gpsimd.dma_start(out=wk_sb, in_=wk[:, :, :])
            nc.gpsimd.dma_start(out=wv_sb, in_=wv[:, :, :])
            nc.gpsimd.dma_start(out=m01_sb, in_=m01[:, :, :])
            nc.gpsimd.dma_start(out=wo_sb, in_=wo[:, :, :])
            nc.gpsimd.dma_start(out=bo2_sb, in_=bo2d[:, :])
            nc.gpsimd.partition_broadcast(bo2_bc[:], bo2_sb[0:1, :])
            # preload the ACT exp table while DMAs are in flight
            warm = const.tile([1, 8], F32, tag="warm")
            nc.vector.memset(warm[:], 1.0)
            nc.scalar.activation(warm[:], warm[:], EXP, scale=0.125)

            def m01_bcast_view(so):
                # [128, (0-stride j=2), 512] broadcast view of m01_sb[:, so, :]
                return bass.AP(
                    tensor=m01_sb.tensor, offset=m01_sb[:, so, 0].offset,
                    ap=[list(m01_sb[:].ap[0]), [0, 2], [1, T]])

            for _rep in range(repeat):
                # ---- activation loads (sync/HWDGE queue)
                x_sbs = []
                for u in range(U):
                    xq_sb = xt_pool.tile([P, KO, T], BF16, tag="xq")
                    xk_sb = xt_pool.tile([P, KO, T], BF16, tag="xk")
                    xv_sb = xt_pool.tile([P, KO, T], BF16, tag="xv")
                    if u == 0:
                        # split the first xq so matmuls start sooner
                        nc.sync.dma_start(out=xq_sb[:, 0:2, :], in_=xqt[u, :, 0:2, :])
                        nc.sync.dma_start(out=xq_sb[:, 2:4, :], in_=xqt[u, :, 2:4, :])
                    else:
                        nc.sync.dma_start(out=xq_sb[:, :, :], in_=xqt[u, :, :, :])
                    nc.sync.dma_start(out=xk_sb[:, :, :], in_=xkt[u, :, :, :])
                    nc.sync.dma_start(out=xv_sb[:, :, :], in_=xvt[u, :, :, :])
                    x_sbs.append((xq_sb, xk_sb, xv_sb))

                # ---- per-unit tiles
                qkv = []
                for u in range(U):
                    qT_sb = qk_pool.tile([P, KO, T], BF16, tag="qT")
                    kT_sb = qk_pool.tile([P, KO, T], BF16, tag="kT")
                    v_sb = qk_pool.tile([P, KO, H * VS], BF16, tag="v")
                    qkv.append((qT_sb, kT_sb, v_sb))

                def P_qk(u):
                    xq_sb, xk_sb, xv_sb = x_sbs[u]
                    qT_sb, kT_sb, v_sb = qkv[u]
                    for mo in range(KO):
                        psq = ps_a.tile([P, T], F32, tag="ps")
                        for ko in range(KO):
                            nc.tensor.matmul(psq[:], wq_sb[:, ko, ts(mo, P)],
                                             xq_sb[:, ko, :],
                                             start=(ko == 0), stop=(ko == KO - 1))
                        nc.scalar.activation(qT_sb[:, mo, :], psq[:], IDENT,
                                             bias=bq_sb[:, mo, None])
                    for mo in range(KO):
                        psk = ps_a.tile([P, T], F32, tag="ps")
                        for ko in range(KO):
                            nc.tensor.matmul(psk[:], wk_sb[:, ko, ts(mo, P)],
                                             xk_sb[:, ko, :],
                                             start=(ko == 0), stop=(ko == KO - 1))
                        nc.scalar.activation(kT_sb[:, mo, :], psk[:], IDENT,
                                             bias=bk_sb[:, mo, None])

                def P_v(u):
                    xq_sb, xk_sb, xv_sb = x_sbs[u]
                    qT_sb, kT_sb, v_sb = qkv[u]
                    # ones columns (position 64 of each 66-wide head block)
                    ones_view = bass.AP(
                        tensor=v_sb.tensor, offset=v_sb[:, 0, 64].offset,
                        ap=[list(v_sb[:].ap[0]), [H * VS, KO], [VS, H], [1, 1]])
                    nc.vector.memset(ones_view, 1.0)
                    for mo in range(KO):
                        psv = ps_a.tile([P, T], F32, tag="ps")
                        for ko in range(KO):
                            nc.tensor.matmul(psv[:], xv_sb[:, ko, ts(mo, P)],
                                             wv_sb[:, ko, :],
                                             start=(ko == 0), stop=(ko == KO - 1))
                        # scatter into per-head 66-strided blocks: [P, H, DK]
                        v_dst = bass.AP(
                            tensor=v_sb.tensor, offset=v_sb[:, mo, 0].offset,
                            ap=[list(v_sb[:].ap[0]), [VS, H], [1, DK]])
                        nc.vector.tensor_copy(out=v_dst,
                                              in_=psv[:].rearrange("p (h d) -> p h d", h=H))

                # ---- attention, spread across the whole iteration: v-proj,
                # the other unit's projections and outproj fill PE time under
                # the ACT exp latency.
                p_tiles = {}
                att_units = []
                for u in range(U):
                    att_units.append(
                        [att_pool.tile([P, T], BF16, tag=f"attT{g}",
                                       name=f"attT{g}_u{u}")
                         for g in range(4)])

                def S(u, g):
                    qT_sb, kT_sb, v_sb = qkv[u]
                    for so in range(KO):
                        sc = ps_sc.tile([P, 2, T], F32, tag="sc")
                        for j in range(2):
                            lo = 64 * j
                            nc.tensor.matmul(
                                sc[:, j, :],
                                kT_sb[lo:lo + 64, g, ts(so, P)],
                                qT_sb[lo:lo + 64, g, :],
                                start=True, stop=True)
                        pt = p_pool.tile([P, 2, T], BF16, tag="p")
                        nc.scalar.activation(pt[:], sc[:], EXP, scale=0.125)
                        # zero masked entries exactly (0/1 mask, both heads)
                        nc.vector.tensor_tensor(out=pt[:], in0=pt[:],
                                                in1=m01_bcast_view(so), op=MUL)
                        p_tiles[(u, g, so)] = pt

                def A(u, g):
                    qT_sb, kT_sb, v_sb = qkv[u]
                    attT_g = att_units[u][g]
                    # pair sums at partition 0 (partition_broadcast requires
                    # partition-0 source AND destination on hardware)
                    s2 = nrm_pool.tile([1, 2, T], F32, tag="s2")
                    avs = []
                    for j, h in enumerate((2 * g, 2 * g + 1)):
                        av = ps_a.tile([P, T], F32, tag="ps", name=f"av{u}{g}{j}")
                        for so in range(KO):
                            nc.tensor.matmul(
                                av[0:65, :],
                                v_sb[:, so, VS * h:VS * h + 65],
                                p_tiles[(u, g, so)][:, j, :],
                                start=(so == 0), stop=(so == KO - 1))
                        nc.scalar.copy(out=s2[0:1, j, :], in_=av[64:65, :])
                        avs.append(av)
                    # one batched in-place reciprocal for the head pair
                    nc.vector.reciprocal_approx_fast(out=s2[:], in_=s2[:])
                    for j in range(2):
                        lo = 64 * j
                        bc = nrm_pool.tile([64, T], F32, tag=f"bc{j}")
                        nc.gpsimd.partition_broadcast(bc[:], s2[0:1, j, :])
                        nc.vector.tensor_tensor(
                            out=attT_g[lo:lo + 64, :],
                            in0=avs[j][0:64, :], in1=bc[:], op=MUL)

                def O(u):
                    attT_k = att_units[u]
                    y_sb = y_pool.tile([P, KO, D], F32, tag="y")
                    for tc_i in range(KO):
                        psy = ps_a.tile([P, T], F32, tag="ps")
                        for ko in range(KO):
                            nc.tensor.matmul(psy[:], attT_k[ko][:, ts(tc_i, P)],
                                             wo_sb[:, ko, :],
                                             start=(ko == 0), stop=(ko == KO - 1))
                        nc.vector.tensor_tensor(out=y_sb[:, tc_i, :], in0=psy[:],
                                                in1=bo2_bc[:], op=ADD)
                    # store on the gpsimd/SWDGE queue so next-iteration x
                    # loads on sync are not blocked behind the store's wait
                    nc.gpsimd.dma_start(out=y[u, :, :, :], in_=y_sb[:])

                # PE filler blocks (v-proj, other-unit proj, outproj) sit
                # between score/av groups to hide the ACT exp latency.
                steps = [
                    lambda: P_qk(0),
                    lambda: S(0, 0),
                    lambda: P_v(0),
                    lambda: S(0, 1),
                    lambda: P_qk(1),
                    lambda: A(0, 0),
                    lambda: S(0, 2),
                    lambda: P_v(1),
                    lambda: A(0, 1),
                    lambda: S(0, 3),
                    lambda: A(0, 2),
                    lambda: S(1, 0),
                    lambda: A(0, 3),
                    lambda: S(1, 1),
                    lambda: O(0),
                    lambda: A(1, 0),
                    lambda: S(1, 2),
                    lambda: A(1, 1),
                    lambda: S(1, 3),
                    lambda: A(1, 2),
                    lambda: A(1, 3),
                    lambda: O(1),
                ]
                for step in steps:
                    step()

    nc.compile()
    return nc


_NC_CACHE = {}


def _get_nc(repeat=1):
    if repeat not in _NC_CACHE:
        _NC_CACHE[repeat] = build_nc(repeat)
    return _NC_CACHE[repeat]


def _bf16():
    import ml_dtypes
    return ml_dtypes.bfloat16


def _chunkT(x):
    """[T,D] -> x^T chunked [P, KO, T] (din = ko*128 + p)."""
    return np.ascontiguousarray(x.T.reshape(KO, P, T).transpose(1, 0, 2))


def _chunkW(w):
    """[D,D] (in,out) -> [P, KO, D]."""
    return np.ascontiguousarray(w.reshape(KO, P, D).transpose(1, 0, 2))


def make_in_maps(query, key, value, mask, Wq, bq, Wk, bk, Wv, bv, Wo, bo):
    bf16 = _bf16()
    query = np.asarray(query, np.float32)
    key = np.asarray(key, np.float32)
    value = np.asarray(value, np.float32)
    mask = np.asarray(mask)
    Wq, Wk, Wv, Wo = (np.asarray(w, np.float32) for w in (Wq, Wk, Wv, Wo))
    bq, bk, bv, bo = (np.asarray(b, np.float32) for b in (bq, bk, bv, bo))

    bo2 = (bv @ Wo + bo).astype(np.float32).reshape(1, D)
    wq_h = _chunkW(Wq).astype(bf16)
    wk_h = _chunkW(Wk).astype(bf16)
    wv_h = _chunkW(Wv).astype(bf16)
    wo_h = _chunkW(Wo).astype(bf16)
    bq_h = np.ascontiguousarray(bq.reshape(KO, P).T)
    bk_h = np.ascontiguousarray(bk.reshape(KO, P).T)

    in_maps = []
    for core in range(NCORES):
        b = core // 4
        cs = [2 * (core % 4), 2 * (core % 4) + 1]
        xq = np.stack([_chunkT(query[b, :, c, :]) for c in cs]).astype(bf16)
        xk = np.stack([_chunkT(key[b, :, c, :]) for c in cs]).astype(bf16)
        xv = np.stack([_chunkT(value[b, :, c, :]) for c in cs]).astype(bf16)
        # 0/1 mask, transposed to [s-part, so, t]
        mb = _chunkT(mask[b, :, :, 0].astype(np.float32)).astype(bf16)
        mb = np.ascontiguousarray(mb)
        in_maps.append({
            "xqt": xq, "xkt": xk, "xvt": xv, "m01": mb,
            "wq": wq_h, "wk": wk_h, "wv": wv_h, "wo": wo_h,
            "bqd": bq_h, "bkd": bk_h, "bo2d": bo2,
        })
    return in_maps


def assemble(results):
    out = np.empty((B, T, C, D), np.float32)
    for core, res in enumerate(results):
        b = core // 4
        cs = [2 * (core % 4), 2 * (core % 4) + 1]
        yv = res["y"]  # [U, P, KO, D]; t = ko*128 + p
        for u, c in enumerate(cs):
            out[b, :, c, :] = yv[u].transpose(1, 0, 2).reshape(T, D)
    return out


def kernel(**inputs):
    from concourse.bass_utils import run_bass_kernel_spmd
    nc = _get_nc()
    in_maps = make_in_maps(**inputs)
    res = run_bass_kernel_spmd(nc, in_maps, core_ids=list(range(NCORES)))
    return assemble(res.results)


if __name__ == "__main__":
    # single-core sim check of core 0 (units b=0, c=0,1)
    import jax
    jax.config.update("jax_platforms", "cpu")
    import sys
    sys.path.insert(0, "/root/problem")
    import reference

    inp = {k: np.asarray(v) for k, v in reference.setup_inputs().items()}
    ref = np.asarray(reference.reference(**inp))

    from concourse.bass_interp import CoreSim
    nc = _get_nc()
    in_maps = make_in_maps(**inp)
    core = 0
    sim = CoreSim(nc)
    sim.assign_tensors(in_maps[core])
    sim.simulate()
    yv = sim.tensor("y")
    b = core // 4
    cs = [2 * (core % 4), 2 * (core % 4) + 1]
    for u, c in enumerate(cs):
        got = yv[u].transpose(1, 0, 2).reshape(T, D)
        want = ref[b, :, c, :]
        err = np.abs(got - want)
        print(f"core0 unit{u} (b={b},c={c}): absmax={err.max():.3e} "
              f"rel={err.max()/np.abs(want).max():.3e}")
